# revision 2
# baseline (speedup 1.0000x reference)
"""Trainium2 Bass kernel for a 2-layer GRU network + sigmoid classifier.

Reference computation (PyTorch-style GRU, gate order r,z,n):
    h1 = GRU0(x);  h2 = GRU1(h1);  out = sigmoid(h2[24] @ W_cls.T + b_cls)

Only h2[24] is consumed, so only timesteps 0..24 of both layers matter.

Strategy (8 NeuronCores, data-parallel over batch: 512 -> 64 per core):
  - Layout: gate/hidden dim on SBUF partitions, batch on the free dim.
  - Gate pre-activations xg = W_ih @ x + bias stay RESIDENT IN SBUF (fp16),
    written chunk-by-chunk by batched projection jobs; no DRAM round trip.
  - Projection jobs (layer-0: xg0 from x; layer-1: xg1 from h1) are woven
    into the PE gaps of the recurrent scans, with forced drains so a
    chunk is always complete before the first scan step that reads it.
  - Per-step recurrence: psum_r / psum_z from W_hh matmuls; psum_n gets
    b_hh_n via a diagonal-matrix matmul (correct PyTorch GRU semantics:
    n = tanh(xn + b_ihn + r*(hn + b_hhn))), then the n-gate chain runs in
    two halves so DVE/ACT pipeline against the PE.
  - SBUF pool scoping: xT/W_ih0 space is released after the last layer-0
    projection job and reused for W_hh1 (DMA'd mid-scan-0).
"""

import numpy as np

SEQ_USED = 25          # classifier reads h2[24]
BATCH = 512
IN_DIM = 512
HID = 768
NCORES = 8
B = BATCH // NCORES    # 64 per core
N = SEQ_USED * B       # 1600 columns in the batched projections
KI = IN_DIM // 128     # 4
KH = HID // 128        # 6
M3 = 3 * HID // 128    # 18 gate row-tiles (r: 0..5, z: 6..11, n: 12..17)
NCH = 4                # batched-projection column chunks
NW = N // NCH          # 400 columns per chunk
NH = KH // 2           # 3 tiles per n-gate half

_CACHE = {}


def _build():
    """Build the SPMD Bass program (identical on all 8 cores)."""
    import concourse.mybir as mybir
    import concourse.tile as tile
    from concourse import bacc

    f32 = mybir.dt.float32
    f16 = mybir.dt.float16
    AF = mybir.ActivationFunctionType

    nc = bacc.Bacc("TRN2", target_bir_lowering=False, debug=False)

    # ---- I/O ----
    xT_d = nc.dram_tensor("xT", [128, KI, N], f16, kind="ExternalInput")
    wih0_d = nc.dram_tensor("wih0", [128, KI, 3 * HID], f16, kind="ExternalInput")
    whh0_d = nc.dram_tensor("whh0", [128, KH, 3 * HID], f16, kind="ExternalInput")
    wih1_d = nc.dram_tensor("wih1", [128, KH, 3 * HID], f16, kind="ExternalInput")
    whh1_d = nc.dram_tensor("whh1", [128, KH, 3 * HID], f16, kind="ExternalInput")
    bias0_d = nc.dram_tensor("bias0", [128, M3], f32, kind="ExternalInput")
    bias1_d = nc.dram_tensor("bias1", [128, M3], f32, kind="ExternalInput")
    diag0_d = nc.dram_tensor("diag0", [128, KH, 128], f16, kind="ExternalInput")
    diag1_d = nc.dram_tensor("diag1", [128, KH, 128], f16, kind="ExternalInput")
    wcls_d = nc.dram_tensor("wcls", [128, KH], f16, kind="ExternalInput")
    bcls_d = nc.dram_tensor("bcls64", [B, 1], f32, kind="ExternalInput")
    y_d = nc.dram_tensor("y", [B, 1], f32, kind="ExternalOutput")

    with tile.TileContext(nc) as tc:
        with (
            tc.tile_pool(name="const", bufs=1) as cpool,
            tc.tile_pool(name="work", bufs=2) as work,
            tc.tile_pool(name="psg", bufs=2, space="PSUM") as psg,
            tc.tile_pool(name="psB", bufs=2, space="PSUM") as psB,
        ):
            # ---- persistent SBUF state ----
            xg = cpool.tile([128, M3, N], f16)      # gate pre-activations
            h1T = cpool.tile([128, KH, N], f16)     # layer-0 outputs
            whh0_sb = cpool.tile([128, KH, 3 * HID], f16)
            wih1_sb = cpool.tile([128, KH, 3 * HID], f16)
            bias0_sb = cpool.tile([128, M3], f32)
            bias1_sb = cpool.tile([128, M3], f32)
            diag0_sb = cpool.tile([128, KH, 128], f16)
            diag1_sb = cpool.tile([128, KH, 128], f16)
            wcls_sb = cpool.tile([128, KH], f16)
            bcls_sb = cpool.tile([B, 1], f32)
            zstate = cpool.tile([128, KH, B], f16)  # h(-1) == 0
            ones64 = cpool.tile([128, B], f16)      # moving operand for diag

            # ---- one GRU step ----
            # xg slices for step t live at columns [t*B, (t+1)*B).
            # hout is a pair of half-slices ([128, NH, B] each).
            def gru_step(t, whh_sb, diag_sb, hprev, hout):
                tc0, tc1 = t * B, (t + 1) * B
                xg_r = xg[:, 0:KH, tc0:tc1]
                xg_z = xg[:, KH:2 * KH, tc0:tc1]

                r16 = work.tile([128, KH, B], f16, tag="r16", name="r16")
                z16 = work.tile([128, KH, B], f16, tag="z16", name="z16")
                if t == 0:
                    # h == 0: gates depend only on xg (biases included)
                    nc.scalar.activation(r16, xg_r, AF.Sigmoid)
                    nc.scalar.activation(z16, xg_z, AF.Sigmoid)
                else:
                    pr = psg.tile([128, KH, B], f32, tag="pr", name="pr")
                    for i in range(KH):
                        m = i * 128
                        for k in range(KH):
                            nc.tensor.matmul(
                                pr[:, i, :], whh_sb[:, k, m:m + 128],
                                hprev[:, k, :],
                                start=(k == 0), stop=(k == KH - 1),
                            )
                    rpre = work.tile([128, KH, B], f16, tag="rpre", name="rpre")
                    nc.vector.tensor_add(rpre, pr, xg_r)
                    nc.scalar.activation(r16, rpre, AF.Sigmoid)

                    pz = psg.tile([128, KH, B], f32, tag="pz", name="pz")
                    for i in range(KH):
                        m = (KH + i) * 128
                        for k in range(KH):
                            nc.tensor.matmul(
                                pz[:, i, :], whh_sb[:, k, m:m + 128],
                                hprev[:, k, :],
                                start=(k == 0), stop=(k == KH - 1),
                            )
                    zpre = work.tile([128, KH, B], f16, tag="zpre", name="zpre")
                    nc.vector.tensor_add(zpre, pz, xg_z)
                    nc.scalar.activation(z16, zpre, AF.Sigmoid)

                # n-gate psum: b_hh_n broadcast (diag matmul) + W_hh_n @ h
                pn = psg.tile([128, KH, B], f32, tag="pn", name="pn")
                for i in range(KH):
                    m = (2 * KH + i) * 128
                    nc.tensor.matmul(
                        pn[:, i, :], diag_sb[:, i, :], ones64,
                        start=True, stop=(t == 0),
                    )
                    if t > 0:
                        for k in range(KH):
                            nc.tensor.matmul(
                                pn[:, i, :], whh_sb[:, k, m:m + 128],
                                hprev[:, k, :],
                                start=False, stop=(k == KH - 1),
                            )

                rhn = work.tile([128, KH, B], f16, tag="rhn", name="rhn")
                npre = work.tile([128, KH, B], f16, tag="npre", name="npre")
                n16 = work.tile([128, KH, B], f16, tag="n16", name="n16")
                d16 = work.tile([128, KH, B], f16, tag="d16", name="d16")
                e16 = work.tile([128, KH, B], f16, tag="e16", name="e16")
                for h in range(2):
                    sl = slice(h * NH, (h + 1) * NH)
                    xg_n = xg[:, 2 * KH + h * NH:2 * KH + (h + 1) * NH, tc0:tc1]
                    nc.vector.tensor_mul(rhn[:, sl], pn[:, sl], r16[:, sl])
                    nc.vector.tensor_add(npre[:, sl], rhn[:, sl], xg_n)
                    nc.scalar.activation(n16[:, sl], npre[:, sl], AF.Tanh)
                # h' = n + z*(h - n), in halves to chase the tanh pipeline
                for h in range(2):
                    sl = slice(h * NH, (h + 1) * NH)
                    nc.vector.tensor_sub(d16[:, sl], hprev[:, sl], n16[:, sl])
                    nc.vector.tensor_mul(e16[:, sl], z16[:, sl], d16[:, sl])
                    nc.vector.tensor_add(hout[h], n16[:, sl], e16[:, sl])
                return z16  # keep handle alive (not otherwise needed)

            # ---- batched projection jobs ----
            def proj_job(w_sb, src_sb, bias_sb, kk, m, c):
                ps = psB.tile([128, NW], f32, tag="psB", name="psB")
                for k in range(kk):
                    nc.tensor.matmul(
                        ps, w_sb[:, k, m * 128:(m + 1) * 128],
                        src_sb[:, k, c * NW:(c + 1) * NW],
                        start=(k == 0), stop=(k == kk - 1),
                    )
                nc.vector.tensor_scalar_add(
                    xg[:, m, c * NW:(c + 1) * NW], ps, bias_sb[:, m:m + 1]
                )

            # chunk-major job lists
            ph1_jobs = [(m, c) for c in range(NCH) for m in range(M3)]
            ph3_jobs = [(m, c) for c in range(NCH) for m in range(M3)]
            ji0 = ji3 = 0
            # layer-1 chunk c is ready once scan-0 has written (and read
            # past) columns [c*NW, (c+1)*NW) -- both happen after step:
            ph3_ready = {0: 6, 1: 12, 2: 18, 3: 24}

            def chunk_needed(t):
                return (t * B + B - 1) // NW

            with tc.tile_pool(name="ph1", bufs=1) as ph1p:
                xT_sb = ph1p.tile([128, KI, N], f16)
                nc.sync.dma_start(xT_sb, xT_d.ap())
                wih0_sb = ph1p.tile([128, KI, 3 * HID], f16)
                nc.sync.dma_start(wih0_sb, wih0_d.ap())
                nc.sync.dma_start(bias0_sb, bias0_d.ap())
                nc.sync.dma_start(diag0_sb, diag0_d.ap())
                nc.sync.dma_start(whh0_sb, whh0_d.ap())
                nc.sync.dma_start(bias1_sb, bias1_d.ap())
                nc.sync.dma_start(diag1_sb, diag1_d.ap())
                nc.sync.dma_start(wih1_sb, wih1_d.ap())
                nc.sync.dma_start(wcls_sb, wcls_d.ap())
                nc.sync.dma_start(bcls_sb, bcls_d.ap())
                nc.vector.memset(zstate, 0.0)
                nc.vector.memset(ones64, 1.0)

                def ph1_job():
                    m, c = ph1_jobs[0]
                    ph1_jobs.pop(0)
                    proj_job(wih0_sb, xT_sb, bias0_sb, KI, m, c)

                # front-load chunk 0 (+ a few of chunk 1)
                for _ in range(22):
                    ph1_job()

                # scan-0 steps 0..16: weave the rest of phase 1
                for t in range(17):
                    while ph1_jobs and ph1_jobs[0][1] <= chunk_needed(t):
                        ph1_job()
                    hprev = zstate if t == 0 else h1T[:, :, (t - 1) * B:t * B]
                    hout = (h1T[:, 0:NH, t * B:(t + 1) * B],
                            h1T[:, NH:KH, t * B:(t + 1) * B])
                    gru_step(t, whh0_sb, diag0_sb, hprev, hout)
                    budget = 3
                    while budget and ph1_jobs:
                        ph1_job()
                        budget -= 1
                while ph1_jobs:  # ensure xT/wih0 are dead before pool close
                    ph1_job()

            with tc.tile_pool(name="late", bufs=1) as latep:
                whh1_sb = latep.tile([128, KH, 3 * HID], f16)
                nc.sync.dma_start(whh1_sb, whh1_d.ap())

                def ph3_job():
                    m, c = ph3_jobs[0]
                    ph3_jobs.pop(0)
                    proj_job(wih1_sb, h1T, bias1_sb, KH, m, c)

                # scan-0 steps 17..24: start weaving layer-1 projections
                for t in range(17, SEQ_USED):
                    hprev = h1T[:, :, (t - 1) * B:t * B]
                    hout = (h1T[:, 0:NH, t * B:(t + 1) * B],
                            h1T[:, NH:KH, t * B:(t + 1) * B])
                    gru_step(t, whh0_sb, diag0_sb, hprev, hout)
                    budget = 2
                    while budget and ph3_jobs and ph3_ready[ph3_jobs[0][1]] <= t:
                        ph3_job()
                        budget -= 1

                # layer-1 scan
                h2prev = zstate
                for t in range(SEQ_USED):
                    while ph3_jobs and ph3_jobs[0][1] <= chunk_needed(t):
                        ph3_job()
                    h2new = work.tile([128, KH, B], f16, tag="h2", name="h2")
                    gru_step(t, whh1_sb, diag1_sb, h2prev,
                             (h2new[:, 0:NH, :], h2new[:, NH:KH, :]))
                    h2prev = h2new
                    budget = 3
                    while budget and ph3_jobs:
                        ph3_job()
                        budget -= 1

                # logits = h2[24].T @ wcls + bcls ; y = sigmoid(logits)
                pc = psB.tile([B, 1], f32, tag="psB", name="pc")
                for k in range(KH):
                    nc.tensor.matmul(
                        pc, h2prev[:, k, :], wcls_sb[:, k:k + 1],
                        start=(k == 0), stop=(k == KH - 1),
                    )
                y_sb = work.tile([B, 1], f32, tag="ysb", name="ysb")
                nc.scalar.activation(y_sb, pc, AF.Sigmoid, bias=bcls_sb)
                nc.sync.dma_start(y_d.ap(), y_sb)

    nc.compile()
    return nc


def _prep_inputs(x, W_ih0, W_hh0, b_ih0, b_hh0, W_ih1, W_hh1, b_ih1, b_hh1,
                 W_cls, b_cls):
    """Shard + relayout the full inputs into per-core in_maps."""
    x = np.asarray(x, np.float32)
    f = lambda a: np.asarray(a, np.float32)

    def kpm(w, kchunks, dt):
        # [3H, D] -> [p, k, m] with w.T reshaped: out[p, k, m] = w[m, k*128+p]
        wt = np.ascontiguousarray(f(w).T)              # [D, 3H]
        return np.ascontiguousarray(
            wt.reshape(kchunks, 128, -1).transpose(1, 0, 2)
        ).astype(dt)

    wih0 = kpm(W_ih0, KI, np.float16)
    whh0 = kpm(W_hh0, KH, np.float16)
    wih1 = kpm(W_ih1, KH, np.float16)
    whh1 = kpm(W_hh1, KH, np.float16)

    def bias_pack(b_ih, b_hh):
        # r,z tiles carry b_ih+b_hh; n tiles carry b_ih only (b_hh_n goes
        # through the diag matmul so it lands inside r*(...)).
        v = np.concatenate([
            (f(b_ih) + f(b_hh))[:2 * HID], f(b_ih)[2 * HID:]
        ])
        return np.ascontiguousarray(v.reshape(M3, 128).T).astype(np.float32)

    def diag_pack(b_hh):
        vn = f(b_hh)[2 * HID:]                          # [768]
        d = np.zeros((128, KH, 128), np.float16)
        idx = np.arange(128)
        for i in range(KH):
            d[idx, i, idx] = vn[i * 128:(i + 1) * 128].astype(np.float16)
        return d

    bias0 = bias_pack(b_ih0, b_hh0)
    bias1 = bias_pack(b_ih1, b_hh1)
    diag0 = diag_pack(b_hh0)
    diag1 = diag_pack(b_hh1)
    wcls = np.ascontiguousarray(
        f(W_cls)[0].reshape(KH, 128).T).astype(np.float16)
    bcls64 = np.full((B, 1), float(np.asarray(b_cls).reshape(-1)[0]), np.float32)

    in_maps = []
    for c in range(NCORES):
        xs = x[:SEQ_USED, c * B:(c + 1) * B, :]        # [25, 64, 512]
        xT = np.ascontiguousarray(
            xs.transpose(2, 0, 1).reshape(KI, 128, N).transpose(1, 0, 2)
        ).astype(np.float16)                            # [128, 4, 1600]
        in_maps.append({
            "xT": xT, "wih0": wih0, "whh0": whh0, "wih1": wih1,
            "whh1": whh1, "bias0": bias0, "bias1": bias1,
            "diag0": diag0, "diag1": diag1,
            "wcls": wcls, "bcls64": bcls64,
        })
    return in_maps


def kernel(**inputs) -> np.ndarray:
    from concourse.bass_utils import run_bass_kernel_spmd

    if "nc" not in _CACHE:
        _CACHE["nc"] = _build()
    nc = _CACHE["nc"]

    in_maps = _prep_inputs(**inputs)
    res = run_bass_kernel_spmd(nc, in_maps, core_ids=list(range(NCORES)))
    outs = [np.asarray(res.results[c]["y"], np.float32) for c in range(NCORES)]
    return np.concatenate(outs, axis=0)          # [512, 1] float32


if __name__ == "__main__":
    rng = np.random.default_rng(0)
    demo = {
        "x": rng.standard_normal((64, BATCH, IN_DIM)).astype(np.float32),
        "W_ih0": rng.standard_normal((3 * HID, IN_DIM)).astype(np.float32) * 0.03,
        "W_hh0": rng.standard_normal((3 * HID, HID)).astype(np.float32) * 0.03,
        "b_ih0": rng.standard_normal(3 * HID).astype(np.float32) * 0.03,
        "b_hh0": rng.standard_normal(3 * HID).astype(np.float32) * 0.03,
        "W_ih1": rng.standard_normal((3 * HID, HID)).astype(np.float32) * 0.03,
        "W_hh1": rng.standard_normal((3 * HID, HID)).astype(np.float32) * 0.03,
        "b_ih1": rng.standard_normal(3 * HID).astype(np.float32) * 0.03,
        "b_hh1": rng.standard_normal(3 * HID).astype(np.float32) * 0.03,
        "W_cls": rng.standard_normal((1, HID)).astype(np.float32) * 0.03,
        "b_cls": rng.standard_normal(1).astype(np.float32) * 0.03,
    }
    print(kernel(**demo)[:8, 0])


# revision 3
# speedup vs baseline: 1.1803x; 1.1803x over previous
"""Trainium2 Bass kernel for a 2-layer GRU network + sigmoid classifier.

Reference computation (PyTorch-style GRU, gate order r,z,n):
    h1 = GRU0(x);  h2 = GRU1(h1);  out = sigmoid(h2[24] @ W_cls.T + b_cls)

Only h2[24] is consumed, so only timesteps 0..24 of both layers matter.

Strategy (8 NeuronCores, data-parallel over batch: 512 -> 64 per core):
  - Layout: gate/hidden dim on SBUF partitions, batch on the free dim.
  - Gate pre-activations xg = W_ih @ x + bias stay RESIDENT IN SBUF (fp16),
    written chunk-by-chunk by batched projection jobs; no DRAM round trip.
  - Projection jobs (layer-0: xg0 from x; layer-1: xg1 from h1) are woven
    into the PE gaps of the recurrent scans, with forced drains so a
    chunk is always complete before the first scan step that reads it.
  - Per-step recurrence: psum_r / psum_z from W_hh matmuls; psum_n gets
    b_hh_n via a diagonal-matrix matmul (correct PyTorch GRU semantics:
    n = tanh(xn + b_ihn + r*(hn + b_hhn))), then the n-gate chain runs in
    two halves so DVE/ACT pipeline against the PE.
  - SBUF pool scoping: xT/W_ih0 space is released after the last layer-0
    projection job and reused for W_hh1 (DMA'd mid-scan-0).
"""

import numpy as np

SEQ_USED = 25          # classifier reads h2[24]
BATCH = 512
IN_DIM = 512
HID = 768
NCORES = 8
B = BATCH // NCORES    # 64 per core
N = SEQ_USED * B       # 1600 columns in the batched projections
KI = IN_DIM // 128     # 4
KH = HID // 128        # 6
M3 = 3 * HID // 128    # 18 gate row-tiles (r: 0..5, z: 6..11, n: 12..17)
NCH = 4                # batched-projection column chunks
NW = N // NCH          # 400 columns per chunk
NH = KH // 2           # 3 tiles per n-gate half

_CACHE = {}


def _build():
    """Build the SPMD Bass program (identical on all 8 cores)."""
    import concourse.mybir as mybir
    import concourse.tile as tile
    from concourse import bacc

    f32 = mybir.dt.float32
    f16 = mybir.dt.float16
    bf16 = mybir.dt.bfloat16
    AF = mybir.ActivationFunctionType

    nc = bacc.Bacc("TRN2", target_bir_lowering=False, debug=False)

    # ---- I/O ----
    xT_d = nc.dram_tensor("xT", [128, KI, N], f16, kind="ExternalInput")
    wih0_d = nc.dram_tensor("wih0", [128, KI, 3 * HID], f16, kind="ExternalInput")
    whh0_d = nc.dram_tensor("whh0", [128, KH, 3 * HID], f16, kind="ExternalInput")
    wih1_d = nc.dram_tensor("wih1", [128, KH, 3 * HID], f16, kind="ExternalInput")
    whh1_d = nc.dram_tensor("whh1", [128, KH, 3 * HID], f16, kind="ExternalInput")
    bias0_d = nc.dram_tensor("bias0", [128, M3], f32, kind="ExternalInput")
    bias1_d = nc.dram_tensor("bias1", [128, M3], f32, kind="ExternalInput")
    diag0_d = nc.dram_tensor("diag0", [128, KH, 128], f16, kind="ExternalInput")
    diag1_d = nc.dram_tensor("diag1", [128, KH, 128], f16, kind="ExternalInput")
    wcls_d = nc.dram_tensor("wcls", [128, KH], f16, kind="ExternalInput")
    bcls_d = nc.dram_tensor("bcls64", [B, 1], f32, kind="ExternalInput")
    y_d = nc.dram_tensor("y", [B, 1], f32, kind="ExternalOutput")

    with tile.TileContext(nc) as tc:
        with (
            tc.tile_pool(name="const", bufs=1) as cpool,
            tc.tile_pool(name="work", bufs=2) as work,
            tc.tile_pool(name="psg", bufs=2, space="PSUM") as psg,
            tc.tile_pool(name="psB", bufs=2, space="PSUM") as psB,
        ):
            # ---- persistent SBUF state ----
            xg = cpool.tile([128, M3, N], bf16)      # gate pre-activations
            h1T = cpool.tile([128, KH, N], bf16)     # layer-0 outputs
            whh0_sb = cpool.tile([128, KH, 3 * HID], f16)
            wih1_sb = cpool.tile([128, KH, 3 * HID], f16)
            bias0_sb = cpool.tile([128, M3], f32)
            bias1_sb = cpool.tile([128, M3], f32)
            diag0_sb = cpool.tile([128, KH, 128], f16)
            diag1_sb = cpool.tile([128, KH, 128], f16)
            wcls_sb = cpool.tile([128, KH], f16)
            bcls_sb = cpool.tile([B, 1], f32)
            zstate = cpool.tile([128, KH, B], bf16)  # h(-1) == 0
            ones64 = cpool.tile([128, B], bf16)      # moving operand for diag

            # ---- one GRU step ----
            # xg slices for step t live at columns [t*B, (t+1)*B).
            # hout is a pair of half-slices ([128, NH, B] each).
            def gru_step(t, whh_sb, diag_sb, hprev, hout):
                tc0, tc1 = t * B, (t + 1) * B
                xg_r = xg[:, 0:KH, tc0:tc1]
                xg_z = xg[:, KH:2 * KH, tc0:tc1]

                r16 = work.tile([128, KH, B], bf16, tag="r16", name="r16")
                z16 = work.tile([128, KH, B], bf16, tag="z16", name="z16")
                if t == 0:
                    # h == 0: gates depend only on xg (biases included)
                    nc.scalar.activation(r16, xg_r, AF.Sigmoid)
                    nc.scalar.activation(z16, xg_z, AF.Sigmoid)
                else:
                    pr = psg.tile([128, KH, B], f32, tag="pr", name="pr")
                    for i in range(KH):
                        m = i * 128
                        for k in range(KH):
                            nc.tensor.matmul(
                                pr[:, i, :], whh_sb[:, k, m:m + 128],
                                hprev[:, k, :],
                                start=(k == 0), stop=(k == KH - 1),
                            )
                    rpre = work.tile([128, KH, B], bf16, tag="rpre", name="rpre")
                    nc.vector.tensor_add(rpre, pr, xg_r)
                    nc.scalar.activation(r16, rpre, AF.Sigmoid)

                    pz = psg.tile([128, KH, B], f32, tag="pz", name="pz")
                    for i in range(KH):
                        m = (KH + i) * 128
                        for k in range(KH):
                            nc.tensor.matmul(
                                pz[:, i, :], whh_sb[:, k, m:m + 128],
                                hprev[:, k, :],
                                start=(k == 0), stop=(k == KH - 1),
                            )
                    zpre = work.tile([128, KH, B], bf16, tag="zpre", name="zpre")
                    nc.vector.tensor_add(zpre, pz, xg_z)
                    nc.scalar.activation(z16, zpre, AF.Sigmoid)

                # n-gate psum: b_hh_n broadcast (diag matmul) + W_hh_n @ h
                pn = psg.tile([128, KH, B], f32, tag="pn", name="pn")
                for i in range(KH):
                    m = (2 * KH + i) * 128
                    nc.tensor.matmul(
                        pn[:, i, :], diag_sb[:, i, :], ones64,
                        start=True, stop=(t == 0),
                    )
                    if t > 0:
                        for k in range(KH):
                            nc.tensor.matmul(
                                pn[:, i, :], whh_sb[:, k, m:m + 128],
                                hprev[:, k, :],
                                start=False, stop=(k == KH - 1),
                            )

                rhn = work.tile([128, KH, B], bf16, tag="rhn", name="rhn")
                npre = work.tile([128, KH, B], bf16, tag="npre", name="npre")
                n16 = work.tile([128, KH, B], bf16, tag="n16", name="n16")
                d16 = work.tile([128, KH, B], bf16, tag="d16", name="d16")
                e16 = work.tile([128, KH, B], bf16, tag="e16", name="e16")
                for h in range(2):
                    sl = slice(h * NH, (h + 1) * NH)
                    xg_n = xg[:, 2 * KH + h * NH:2 * KH + (h + 1) * NH, tc0:tc1]
                    nc.vector.tensor_mul(rhn[:, sl], pn[:, sl], r16[:, sl])
                    nc.vector.tensor_add(npre[:, sl], rhn[:, sl], xg_n)
                    nc.scalar.activation(n16[:, sl], npre[:, sl], AF.Tanh)
                # h' = n + z*(h - n), in halves to chase the tanh pipeline
                for h in range(2):
                    sl = slice(h * NH, (h + 1) * NH)
                    nc.vector.tensor_sub(d16[:, sl], hprev[:, sl], n16[:, sl])
                    nc.vector.tensor_mul(e16[:, sl], z16[:, sl], d16[:, sl])
                    nc.vector.tensor_add(hout[h], n16[:, sl], e16[:, sl])
                return z16  # keep handle alive (not otherwise needed)

            # ---- batched projection jobs ----
            def proj_job(w_sb, src_sb, bias_sb, kk, m, c):
                ps = psB.tile([128, NW], f32, tag="psB", name="psB")
                for k in range(kk):
                    nc.tensor.matmul(
                        ps, w_sb[:, k, m * 128:(m + 1) * 128],
                        src_sb[:, k, c * NW:(c + 1) * NW],
                        start=(k == 0), stop=(k == kk - 1),
                    )
                nc.vector.tensor_scalar_add(
                    xg[:, m, c * NW:(c + 1) * NW], ps, bias_sb[:, m:m + 1]
                )

            # chunk-major job lists
            ph1_jobs = [(m, c) for c in range(NCH) for m in range(M3)]
            ph3_jobs = [(m, c) for c in range(NCH) for m in range(M3)]
            ji0 = ji3 = 0
            # layer-1 chunk c is ready once scan-0 has written (and read
            # past) columns [c*NW, (c+1)*NW) -- both happen after step:
            ph3_ready = {0: 6, 1: 12, 2: 18, 3: 24}

            def chunk_needed(t):
                return (t * B + B - 1) // NW

            with tc.tile_pool(name="ph1", bufs=1) as ph1p:
                xT_sb = ph1p.tile([128, KI, N], f16)
                nc.sync.dma_start(xT_sb, xT_d.ap())
                wih0_sb = ph1p.tile([128, KI, 3 * HID], f16)
                nc.sync.dma_start(wih0_sb, wih0_d.ap())
                nc.sync.dma_start(bias0_sb, bias0_d.ap())
                nc.sync.dma_start(diag0_sb, diag0_d.ap())
                nc.sync.dma_start(whh0_sb, whh0_d.ap())
                nc.sync.dma_start(bias1_sb, bias1_d.ap())
                nc.sync.dma_start(diag1_sb, diag1_d.ap())
                nc.sync.dma_start(wih1_sb, wih1_d.ap())
                nc.sync.dma_start(wcls_sb, wcls_d.ap())
                nc.sync.dma_start(bcls_sb, bcls_d.ap())
                nc.vector.memset(zstate, 0.0)
                nc.vector.memset(ones64, 1.0)

                def ph1_job():
                    m, c = ph1_jobs[0]
                    ph1_jobs.pop(0)
                    proj_job(wih0_sb, xT_sb, bias0_sb, KI, m, c)

                # front-load chunk 0 (+ a few of chunk 1)
                for _ in range(18):
                    ph1_job()

                # scan-0 steps 0..16: weave the rest of phase 1
                for t in range(17):
                    while ph1_jobs and ph1_jobs[0][1] <= chunk_needed(t):
                        ph1_job()
                    hprev = zstate if t == 0 else h1T[:, :, (t - 1) * B:t * B]
                    hout = (h1T[:, 0:NH, t * B:(t + 1) * B],
                            h1T[:, NH:KH, t * B:(t + 1) * B])
                    gru_step(t, whh0_sb, diag0_sb, hprev, hout)
                    budget = 3
                    while budget and ph1_jobs:
                        ph1_job()
                        budget -= 1
                while ph1_jobs:  # ensure xT/wih0 are dead before pool close
                    ph1_job()

            with tc.tile_pool(name="late", bufs=1) as latep:
                whh1_sb = latep.tile([128, KH, 3 * HID], f16)
                nc.sync.dma_start(whh1_sb, whh1_d.ap())

                def ph3_job():
                    m, c = ph3_jobs[0]
                    ph3_jobs.pop(0)
                    proj_job(wih1_sb, h1T, bias1_sb, KH, m, c)

                # scan-0 steps 17..24: start weaving layer-1 projections
                for t in range(17, SEQ_USED):
                    hprev = h1T[:, :, (t - 1) * B:t * B]
                    hout = (h1T[:, 0:NH, t * B:(t + 1) * B],
                            h1T[:, NH:KH, t * B:(t + 1) * B])
                    gru_step(t, whh0_sb, diag0_sb, hprev, hout)
                    budget = 2
                    while budget and ph3_jobs and ph3_ready[ph3_jobs[0][1]] <= t:
                        ph3_job()
                        budget -= 1

                # layer-1 scan
                h2prev = zstate
                for t in range(SEQ_USED):
                    while ph3_jobs and ph3_jobs[0][1] <= chunk_needed(t):
                        ph3_job()
                    h2new = work.tile([128, KH, B], bf16, tag="h2", name="h2")
                    gru_step(t, whh1_sb, diag1_sb, h2prev,
                             (h2new[:, 0:NH, :], h2new[:, NH:KH, :]))
                    h2prev = h2new
                    budget = 2
                    while budget and ph3_jobs:
                        ph3_job()
                        budget -= 1

                # logits = h2[24].T @ wcls + bcls ; y = sigmoid(logits)
                pc = psB.tile([B, 1], f32, tag="psB", name="pc")
                for k in range(KH):
                    nc.tensor.matmul(
                        pc, h2prev[:, k, :], wcls_sb[:, k:k + 1],
                        start=(k == 0), stop=(k == KH - 1),
                    )
                y_sb = work.tile([B, 1], f32, tag="ysb", name="ysb")
                nc.scalar.activation(y_sb, pc, AF.Sigmoid, bias=bcls_sb)
                nc.sync.dma_start(y_d.ap(), y_sb)

    nc.compile()
    return nc


def _prep_inputs(x, W_ih0, W_hh0, b_ih0, b_hh0, W_ih1, W_hh1, b_ih1, b_hh1,
                 W_cls, b_cls):
    """Shard + relayout the full inputs into per-core in_maps."""
    x = np.asarray(x, np.float32)
    f = lambda a: np.asarray(a, np.float32)

    def kpm(w, kchunks, dt):
        # [3H, D] -> [p, k, m] with w.T reshaped: out[p, k, m] = w[m, k*128+p]
        wt = np.ascontiguousarray(f(w).T)              # [D, 3H]
        return np.ascontiguousarray(
            wt.reshape(kchunks, 128, -1).transpose(1, 0, 2)
        ).astype(dt)

    wih0 = kpm(W_ih0, KI, np.float16)
    whh0 = kpm(W_hh0, KH, np.float16)
    wih1 = kpm(W_ih1, KH, np.float16)
    whh1 = kpm(W_hh1, KH, np.float16)

    def bias_pack(b_ih, b_hh):
        # r,z tiles carry b_ih+b_hh; n tiles carry b_ih only (b_hh_n goes
        # through the diag matmul so it lands inside r*(...)).
        v = np.concatenate([
            (f(b_ih) + f(b_hh))[:2 * HID], f(b_ih)[2 * HID:]
        ])
        return np.ascontiguousarray(v.reshape(M3, 128).T).astype(np.float32)

    def diag_pack(b_hh):
        vn = f(b_hh)[2 * HID:]                          # [768]
        d = np.zeros((128, KH, 128), np.float16)
        idx = np.arange(128)
        for i in range(KH):
            d[idx, i, idx] = vn[i * 128:(i + 1) * 128].astype(np.float16)
        return d

    bias0 = bias_pack(b_ih0, b_hh0)
    bias1 = bias_pack(b_ih1, b_hh1)
    diag0 = diag_pack(b_hh0)
    diag1 = diag_pack(b_hh1)
    wcls = np.ascontiguousarray(
        f(W_cls)[0].reshape(KH, 128).T).astype(np.float16)
    bcls64 = np.full((B, 1), float(np.asarray(b_cls).reshape(-1)[0]), np.float32)

    in_maps = []
    for c in range(NCORES):
        xs = x[:SEQ_USED, c * B:(c + 1) * B, :]        # [25, 64, 512]
        xT = np.ascontiguousarray(
            xs.transpose(2, 0, 1).reshape(KI, 128, N).transpose(1, 0, 2)
        ).astype(np.float16)                            # [128, 4, 1600]
        in_maps.append({
            "xT": xT, "wih0": wih0, "whh0": whh0, "wih1": wih1,
            "whh1": whh1, "bias0": bias0, "bias1": bias1,
            "diag0": diag0, "diag1": diag1,
            "wcls": wcls, "bcls64": bcls64,
        })
    return in_maps


def kernel(**inputs) -> np.ndarray:
    from concourse.bass_utils import run_bass_kernel_spmd

    if "nc" not in _CACHE:
        _CACHE["nc"] = _build()
    nc = _CACHE["nc"]

    in_maps = _prep_inputs(**inputs)
    res = run_bass_kernel_spmd(nc, in_maps, core_ids=list(range(NCORES)))
    outs = [np.asarray(res.results[c]["y"], np.float32) for c in range(NCORES)]
    return np.concatenate(outs, axis=0)          # [512, 1] float32


if __name__ == "__main__":
    rng = np.random.default_rng(0)
    demo = {
        "x": rng.standard_normal((64, BATCH, IN_DIM)).astype(np.float32),
        "W_ih0": rng.standard_normal((3 * HID, IN_DIM)).astype(np.float32) * 0.03,
        "W_hh0": rng.standard_normal((3 * HID, HID)).astype(np.float32) * 0.03,
        "b_ih0": rng.standard_normal(3 * HID).astype(np.float32) * 0.03,
        "b_hh0": rng.standard_normal(3 * HID).astype(np.float32) * 0.03,
        "W_ih1": rng.standard_normal((3 * HID, HID)).astype(np.float32) * 0.03,
        "W_hh1": rng.standard_normal((3 * HID, HID)).astype(np.float32) * 0.03,
        "b_ih1": rng.standard_normal(3 * HID).astype(np.float32) * 0.03,
        "b_hh1": rng.standard_normal(3 * HID).astype(np.float32) * 0.03,
        "W_cls": rng.standard_normal((1, HID)).astype(np.float32) * 0.03,
        "b_cls": rng.standard_normal(1).astype(np.float32) * 0.03,
    }
    print(kernel(**demo)[:8, 0])
